# revision 26
# baseline (speedup 1.0000x reference)
"""Fused AllReduce + residual-add + RMSNorm kernel for one TRN2 chip (8 NeuronCores).

Reference computation (for full input [tp=8, tokens=4096, hidden=4096] f32):
    reduced = input.sum(axis=0)
    hidden  = reduced + residual
    norm    = hidden * rsqrt(mean(hidden^2, -1) + 1e-6) * norm_weight
    return (norm, hidden)

Sharding strategy: shard the TOKEN axis, not the tp axis. Core c receives
input[:, c*512:(c+1)*512, :] -- all 8 partial sums for its 512 tokens -- and
does a purely local 8-way sum + residual + RMSNorm. No collective needed,
perfect parallelism, and total HBM traffic equals the unavoidable minimum
(~88MB per core, ~246us at the 358GB/s per-core HBM limit).

Per-core pipeline (4 token-tiles of 128 tokens x 4096 hidden):
  - DMA: residual tile + 4x 2-slab input groups (4MB transfers, HWDGE).
  - TensorE: 9 identity-matmuls per PSUM bank accumulate res + 8 slabs into
    PSUM (float32r -> full-rate streaming).
  - ScalarE: copy PSUM->SBUF (hidden), Square+accum_out for sum(h^2) in
    place on the dead PSUM, Sqrt for the rstd.
  - VectorE: reciprocal + the two norm multiplies (x w first -- it does not
    depend on rstd, shortening the stats critical path).
  - DMA out: hidden + norm tiles.
"""

import numpy as np

import concourse.bass as bass
import concourse.tile as tile
from concourse import bacc, mybir
from concourse.bass_utils import run_bass_kernel_spmd
from concourse.tile import add_dep_helper

TP = 8
TOKENS = 4096
HIDDEN = 4096
N_CORES = 8
TOK_PER_CORE = TOKENS // N_CORES  # 512
P = 128  # SBUF partitions
N_TILES = TOK_PER_CORE // P  # 4 token-tiles per core
EPS = 1e-6
F32 = mybir.dt.float32
F32R = mybir.dt.float32r
NB = HIDDEN // 512  # PSUM banks per tile (8)
GRP = 2  # input slabs per DMA group


def _build():
    nc = bacc.Bacc("TRN2")
    x_ext = nc.declare_dram_parameter(
        "input", [TP, TOK_PER_CORE, HIDDEN], F32R, isOutput=False
    )
    r_ext = nc.declare_dram_parameter(
        "residual", [TOK_PER_CORE, HIDDEN], F32R, isOutput=False
    )
    w_ext = nc.declare_dram_parameter("norm_weight", [HIDDEN], F32R, isOutput=False)
    norm_ext = nc.declare_dram_parameter(
        "norm", [TOK_PER_CORE, HIDDEN], F32, isOutput=True
    )
    hid_ext = nc.declare_dram_parameter(
        "hidden", [TOK_PER_CORE, HIDDEN], F32, isOutput=True
    )
    id_ext = nc.declare_dram_parameter("ident", [P, P], F32R, isOutput=False)
    ones_ext = nc.declare_dram_parameter("ones", [1, P], F32R, isOutput=False)

    with tile.TileContext(nc) as tc:
        with (
            tc.tile_pool(name="singles", bufs=1) as singles,
            tc.tile_pool(name="xsp", bufs=3) as xsp,
            tc.tile_pool(name="resp", bufs=1) as resp,
            tc.tile_pool(name="hidp", bufs=1) as hidp,
            tc.tile_pool(name="normp", bufs=3) as normp,
            tc.tile_pool(name="statsp", bufs=2) as statsp,
            tc.tile_pool(name="psump", bufs=1, space="PSUM") as psump,
        ):
            ident = singles.tile([P, P], F32R)
            nc.gpsimd.dma_start(out=ident, in_=id_ext[:, :])

            # norm_weight broadcast to all 128 partitions via PE ones-matmul
            # (reads 16KB from HBM once instead of 128x)
            ones_t = singles.tile([1, P], F32R)
            nc.gpsimd.dma_start(out=ones_t, in_=ones_ext[:, :])
            w_sb = normp.tile([1, HIDDEN], F32R, tag="nt")
            nc.gpsimd.dma_start(out=w_sb, in_=w_ext[:].rearrange("(o h) -> o h", o=1))
            w_b = singles.tile([P, HIDDEN], F32)
            psum_w = psump.tile([P, HIDDEN], F32, tag="ps")
            for b in range(NB):
                nc.tensor.matmul(
                    psum_w[:, b * 512 : (b + 1) * 512],
                    ones_t,
                    w_sb[:, b * 512 : (b + 1) * 512],
                    start=True,
                    stop=True,
                )
            nc.scalar.copy(out=w_b, in_=psum_w)
            eps_t = singles.tile([P, 1], F32)
            nc.vector.memset(eps_t, EPS)

            norm_dmas = []
            dep_input_dma = None

            for it in range(N_TILES):
                t0 = it * P
                res_t = resp.tile([P, HIDDEN], F32R, tag="res")
                nc.sync.dma_start(out=res_t, in_=r_ext[t0 : t0 + P, :])
                xs_tiles = []
                for g in range(TP // GRP):
                    xs = xsp.tile([P, GRP, HIDDEN], F32R, tag="xs")
                    src = x_ext[g * GRP : (g + 1) * GRP, t0 : t0 + P, :].rearrange(
                        "p t h -> t p h"
                    )
                    d = nc.sync.dma_start(out=xs, in_=src)
                    if it == N_TILES - 1 and g == 2:
                        dep_input_dma = d
                    xs_tiles.append(xs)

                # PSUM accumulate: res + 8 slabs, via identity matmul (f32r)
                psum_t = psump.tile([P, HIDDEN], F32, tag="ps")
                for b in range(NB):
                    nc.tensor.matmul(
                        psum_t[:, b * 512 : (b + 1) * 512],
                        ident,
                        res_t[:, b * 512 : (b + 1) * 512],
                        start=True,
                        stop=False,
                    )
                for g in range(TP // GRP):
                    for j in range(GRP):
                        last = g == TP // GRP - 1 and j == GRP - 1
                        for b in range(NB):
                            nc.tensor.matmul(
                                psum_t[:, b * 512 : (b + 1) * 512],
                                ident,
                                xs_tiles[g][:, j, b * 512 : (b + 1) * 512],
                                start=False,
                                stop=last,
                            )

                # hidden = PSUM -> SBUF via ScalarE, then DMA out
                hid_t = hidp.tile([P, HIDDEN], F32, tag="hid")
                nc.scalar.copy(out=hid_t, in_=psum_t)
                nc.scalar.dma_start(out=hid_ext[t0 : t0 + P, :], in_=hid_t)

                # sum(h^2) via ACT Square with accum_out (in-place on PSUM,
                # which is dead after the copy above)
                msq = statsp.tile([P, 1], F32, tag="msq")
                nc.scalar.activation(
                    out=psum_t,
                    in_=psum_t,
                    func=mybir.ActivationFunctionType.Square,
                    accum_out=msq,
                )
                rstd = statsp.tile([P, 1], F32, tag="rstd")
                nc.scalar.activation(
                    out=rstd,
                    in_=msq,
                    func=mybir.ActivationFunctionType.Sqrt,
                    bias=eps_t,
                    scale=1.0 / HIDDEN,
                )
                nc.vector.reciprocal(out=rstd, in_=rstd)

                # norm = (hidden * w) * rstd -- w-mul first: it does not
                # depend on rstd, so it overlaps the stats chain
                nt = normp.tile([P, HIDDEN], F32, tag="nt")
                nc.vector.tensor_mul(out=nt, in0=hid_t, in1=w_b)
                nc.vector.tensor_scalar_mul(out=nt, in0=nt, scalar1=rstd)
                norm_dmas.append(nc.gpsimd.dma_start(out=norm_ext[t0 : t0 + P, :], in_=nt))

            # Defer the norm stores until the whole input stream has been
            # fetched: the end-of-input window (last tile's matmul+stats
            # chain) then gets filled with the norm-store backlog instead of
            # idling the DMA engines.
            for nd in norm_dmas[:-1]:
                add_dep_helper(nd.ins, dep_input_dma.ins, reason="defer norm stores past input stream")

    nc.finalize()  # Bacc: runs compile passes (event-sem split, reg alloc)
    return nc


_NC = None


def _get_nc():
    global _NC
    if _NC is None:
        _NC = _build()
    return _NC


def _run(input, residual, norm_weight, trace=False):
    input = np.ascontiguousarray(np.asarray(input), dtype=np.float32)
    residual = np.ascontiguousarray(np.asarray(residual), dtype=np.float32)
    norm_weight = np.ascontiguousarray(np.asarray(norm_weight), dtype=np.float32)

    in_maps = []
    for c in range(N_CORES):
        t0 = c * TOK_PER_CORE
        in_maps.append(
            {
                "input": np.ascontiguousarray(input[:, t0 : t0 + TOK_PER_CORE, :]),
                "residual": np.ascontiguousarray(residual[t0 : t0 + TOK_PER_CORE, :]),
                "norm_weight": norm_weight,
                "ident": np.eye(P, dtype=np.float32),
                "ones": np.ones((1, P), dtype=np.float32),
            }
        )
    res = run_bass_kernel_spmd(
        _get_nc(), in_maps, core_ids=list(range(N_CORES)), trace=trace
    )
    outs = res.results
    norm = np.concatenate([outs[c]["norm"] for c in range(N_CORES)], axis=0)
    hidden = np.concatenate([outs[c]["hidden"] for c in range(N_CORES)], axis=0)
    return (norm, hidden), res


def kernel(input, residual, norm_weight):
    (norm, hidden), _ = _run(input, residual, norm_weight, trace=False)
    return norm, hidden
